# revision 13
# baseline (speedup 1.0000x reference)
"""Trainium2 Bass kernel for nn_CausalAttention (no actual causal mask, per the
reference bug): out = softmax((x@Wq)(x@Wk)^T / 64**0.05) @ (x@Wv).

Sharding: data-parallel over batch, one batch element per NeuronCore (B=8).

Key structure (v2 — dual-engine softmax + phase overlap):
 - Scores are produced in "bf16-bit units": W_q is pre-scaled on the HOST by
   128*log2(e)/SCALE, so the QK^T PSUM value s_b satisfies
   exp(s/SCALE) = 2^(s_b/128).  The exp is then computed on TWO engines
   concurrently (windows split between them):
     * ACT (scalar) engine: exact spline exp with scale=ln2/128, bias=-25.
     * DVE (vector) engine: a CUSTOM 8-stage ALU op (EXP2_BITS_ANT) that
       computes the bf16 BIT PATTERN of 2^(s_b/128 + c) directly:
       magic-number floor-split + quadratic mantissa-hump correction,
       written as uint16 and bit-viewed as bf16 (max rel err ~0.6%, on par
       with the bf16 quantization the exact path pays anyway).
   The scalar engine alone was the phase-2 bottleneck (100% busy at 205us).
 - 2-deep software pipeline: the PE FIFO runs QK(i+2) before PV(i), so
   exp(i) and exp(i+1) run concurrently on the two engines while the PE
   streams the previous window's PV.
 - Phase 1 (x^T DMA transposes + q/k/v projections) is interleaved into the
   first q-chunk pair's windows so the serialized xbar transposes (~1.35us
   per [1024,128] fp16 slab, one safe DMA ring) hide behind attention.
   The first s-chunk is 512 rows to shorten the startup ramp.
 - q^T/k^T are produced in ONE projection pass with stationary [Wq|Wk]
   (M=128); PSUM->SBUF copies go to the SCALAR engine, and the swapped-half
   copy needed for row-paired QK^T matmuls (two K=64 matmuls in the two PE
   row halves via tile_position) runs on GPSIMD — the DVE stays free for exp.
 - v_aug is padded to M=128 ([v | ones | zeros]) so PV matmuls are
   full-array (K=128, M=128) and keep the PE HAM clock-gate at 2.4 GHz
   without heater matmuls.
 - probabilities bf16, x fp16 (host-preformatted feature-chunk-major for the
   2-byte xbar DMA transpose), all matmuls accumulate in fp32 PSUM; softmax
   denominator comes free via the ones column (sum of the same rounded
   weights -> output stays a proper weighted average).
"""

import sys

import numpy as np

for _p in ("/root/.axon_site", "/root/.axon_site/_ro/trn_rl_repo",
           "/root/.axon_site/_ro/pypackages", "/opt/trn_rl_repo"):
    if _p not in sys.path:
        sys.path.append(_p)

B, S, D, H = 8, 4096, 768, 64
P = 128
SCALE = float(H) ** 0.05
LOG2E = 1.4426950408889634
QBIT = 128.0 * LOG2E / SCALE      # host pre-scale folded into W_q
EXP_SHIFT = -25.0                 # common shift, cancels in softmax

# EXP2_BITS_ANT constants (fp32->uint16 cast rounds to nearest, HW-verified)
E2_C2 = 0.00265                                      # mantissa-hump quad coef
E2_C0 = 128.0 * (127.0 + EXP_SHIFT * LOG2E) - 64.0   # bias - 64 (floor split)
E2_C1 = 1.5 * 2.0 ** 30                              # magic (rounds to k*128)
E2_C3 = (64.0 - 4096.0 * E2_C2) / E2_C2              # alignment const / coef

_cached = {}


def _register_exp2():
    """Register the custom DVE op computing bf16 bits of 2^((x+C0)/128)+hump.

    bits = Y2 + ((F*F) + C3)*C2 ; Y2 = x + C0 ; F = Y2 - round128(Y2)
    Exactly 8 ALU stages; C3 spilled via in1 per the custom-DVE API.
    """
    from concourse import dve_ops as dvo
    from concourse.dve_spec import (
        C0, C1, C2, C3, Spec, Src0, _spill_c3_to_src1, lower,
    )
    from concourse.dve_uop import DveOpSpec

    for op in dvo.OPS:
        if op.name == "EXP2_BITS_ANT":
            return op

    Y2 = Src0 + C0
    T = Y2 + C1
    N = T - C1
    F = Y2 - N
    Q = F * F
    bits = Y2 + (Q + C3) * C2

    def ref(in0, in1, c0, c1, c2):
        y2 = in0.astype(np.float32) + np.float32(c0)
        t = (y2 + np.float32(c1)).astype(np.float32)
        n = (t - np.float32(c1)).astype(np.float32)
        f = y2 - n
        c3 = np.asarray(in1, np.float32).reshape(in0.shape[0], 1)
        return ((f * f) + c3) * np.float32(c2) + y2

    spec = Spec(body=_spill_c3_to_src1(bits), reference=ref)
    shas = {}
    for ver in ("v3", "v4"):
        uops = lower(spec, ver=ver)
        shas[ver] = DveOpSpec(
            name="EXP2_BITS_ANT", opcode=None, uops=uops, rd1_en=True
        ).sha(ver)
    op = dvo.DveOp("EXP2_BITS_ANT", spec, subdim=False, uops_sha=shas)
    dvo.OPS.append(op)
    dvo.CUSTOM_DVE_SPECS[op.name] = op.spec
    dvo._SUB_OPCODE_FOR_NAME[op.name] = dvo._CUSTOM_DVE_ROW_BASE + len(dvo.OPS) - 1
    return op


def build_program(S=S, D=D, H=H, QC=512, WIN=2, dve_num=1, dve_den=2,
                  heater=False, va=P, qk_copy_eng="scalar", swap_eng="gpsimd"):
    import concourse.mybir as mybir
    import concourse.tile as tile
    from concourse import bacc
    from concourse.masks import make_identity

    EXP2 = _register_exp2()

    NF = D // P          # feature chunks (6)
    KC = S // P          # k-chunks (32)
    NQC = S // QC        # q-chunks (8)
    VA = va              # padded v_aug width (full-array PV)
    # phase-1 s-chunks (rows); first ones smaller for a fast startup ramp
    CH_ROWS = [512, 512, 1024, 1024, 1024]
    assert sum(CH_ROWS) == S
    CH_K = [r // P for r in CH_ROWS]          # k-chunks per p1 chunk
    CH_OFF = [sum(CH_ROWS[:i]) for i in range(len(CH_ROWS))]

    f32 = mybir.dt.float32
    f16 = mybir.dt.float16
    bf16 = mybir.dt.bfloat16
    u16 = mybir.dt.uint16

    nc = bacc.Bacc("TRN2", target_bir_lowering=False)

    x_d = nc.dram_tensor("x16", [NF, S, P], f16, kind="ExternalInput")
    wq_d = nc.dram_tensor("wq", [D, H], f32, kind="ExternalInput")  # pre-scaled
    wk_d = nc.dram_tensor("wk", [D, H], f32, kind="ExternalInput")
    wv_d = nc.dram_tensor("wv", [D, H], f32, kind="ExternalInput")
    out_d = nc.dram_tensor("out", [S, H], f32, kind="ExternalOutput")

    with tile.TileContext(nc) as tc:
        with (
            tc.tile_pool(name="persist", bufs=1) as persist,
            tc.tile_pool(name="xts", bufs=2) as xts,
            tc.tile_pool(name="ptp", bufs=4) as ptp,
            tc.tile_pool(name="drainp", bufs=2) as drainp,
            tc.tile_pool(name="stp", bufs=3, space="PSUM") as stp,
            tc.tile_pool(name="op", bufs=2, space="PSUM") as opp,
        ):
            # [q (0:64) ; k (64:128)] on partitions, s on free dim
            qkT = persist.tile([P, S], f16)
            qkTs = persist.tile([P, S], f16)      # halves swapped: [k ; q]
            v_aug = persist.tile([P, KC, VA], f16)  # [kpart, chunk, v|1|0pad]
            w_stage = persist.tile([P, 3, NF, H], f32)
            wqk_sb = persist.tile([P, NF, P], f16)  # [ Wq | Wk ] per chunk
            wv_sb = persist.tile([P, NF, H], f16)
            ident = persist.tile([P, P], f32)
            ident16 = persist.tile([P, P], f16)
            exp_bias = persist.tile([P, 1], f32)
            c3t = persist.tile([P, 1], f32)
            heat = persist.tile([P, P], f16)

            make_identity(nc, ident)
            make_identity(nc, ident16)
            if VA > H + 1:
                nc.vector.memset(v_aug[:, :, H:VA], 0.0)
            nc.vector.memset(v_aug[:, :, H:H + 1], 1.0)
            nc.vector.memset(exp_bias, EXP_SHIFT)
            nc.vector.memset(c3t, E2_C3)
            if heater:
                nc.vector.memset(heat, 0.001)
            for i, w_d in enumerate((wq_d, wk_d, wv_d)):
                nc.sync.dma_start(
                    w_stage[:, i], w_d[:].rearrange("(g p) h -> p g h", p=P)
                )
            nc.vector.tensor_copy(wqk_sb[:, :, 0:H], w_stage[:, 0])
            nc.vector.tensor_copy(wqk_sb[:, :, H:P], w_stage[:, 1])
            nc.vector.tensor_copy(wv_sb[:], w_stage[:, 2])

            qk_copy = nc.scalar.copy if qk_copy_eng == "scalar" else \
                (lambda o, i_: nc.vector.tensor_copy(o, i_))
            swap_copy = nc.gpsimd.tensor_copy if swap_eng == "gpsimd" else \
                nc.vector.tensor_copy

            # ---------------- phase-1 pieces ----------------
            xf_tiles = {}

            def p1_dma(c):
                rows = CH_ROWS[c]
                sl = slice(CH_OFF[c], CH_OFF[c] + rows)
                xf = xts.tile([P, NF, 1024], f16, tag="xf", name="xf")
                for g in range(NF):
                    nc.sync.dma_start_transpose(xf[:, g, 0:rows], x_d[g, sl, :])
                xf_tiles[c] = xf

            qk_ps = {}

            def p1_qk_mm(c, half):
                xf = xf_tiles[c]
                hs = slice(half * 512, (half + 1) * 512)
                ps = stp.tile([P, WIN, QC], f32, tag="st", name="ps")
                psf = ps.rearrange("p a b -> p (a b)")
                for g in range(NF):
                    nc.tensor.matmul(
                        psf[:, 0:512], wqk_sb[:, g], xf[:, g, hs],
                        start=(g == 0), stop=(g == NF - 1),
                    )
                qk_ps[(c, half)] = psf

            def p1_qk_copy(c, half):
                psf = qk_ps.pop((c, half))
                col0 = CH_OFF[c] + half * 512
                cols = slice(col0, col0 + 512)
                qk_copy(qkT[:, cols], psf[:, 0:512])
                # swapped halves [k ; q] for the paired QK^T matmuls
                swap_copy(qkTs[0:H, cols], qkT[H:P, cols])
                swap_copy(qkTs[H:P, cols], qkT[0:H, cols])

            v_ps = {}

            def p1_v_mm(c):
                # v^T = Wv^T x^T with the CONSTANT wv stationary: 6 stream-
                # bound matmuls instead of 48 LDW-bound ones per chunk.
                xf = xf_tiles[c]
                rows = CH_ROWS[c]
                ps = stp.tile([P, WIN, QC], f32, tag="st", name="psvt")
                psf = ps.rearrange("p a b -> p (a b)")
                for half in range(rows // 512):
                    hs = slice(half * 512, (half + 1) * 512)
                    for g in range(NF):
                        nc.tensor.matmul(
                            psf[0:H, hs], wv_sb[:, g], xf[:, g, hs],
                            start=(g == 0), stop=(g == NF - 1),
                        )
                v_ps[c] = psf
                xf_tiles.pop(c)

            def p1_v_cp1(c):
                psf = v_ps.pop(c)
                rows = CH_ROWS[c]
                vt = drainp.tile([H, 1024], f16, tag="vts", name="vt")
                nc.scalar.copy(vt[:, 0:rows], psf[0:H, 0:rows])
                v_ps[c] = vt

            def p1_v_tr(c):
                vt = v_ps.pop(c)
                kpc = CH_K[c]
                ps2 = stp.tile([P, WIN * QC], f16, tag="st", name="psv2")
                for t in range(kpc):
                    nc.tensor.transpose(
                        ps2[:, t * H:(t + 1) * H],
                        vt[:, t * P:(t + 1) * P],
                        ident16[0:H, 0:H],
                    )
                v_ps[c] = ps2

            def p1_v_cp2(c):
                ps2 = v_ps.pop(c)
                kpc = CH_K[c]
                k0 = CH_OFF[c] // P
                src_v = ps2[:, 0:kpc * H].rearrange("p (t h) -> p t h", h=H)
                nc.vector.tensor_copy(v_aug[:, k0:k0 + kpc, 0:H], src_v)

            def p1_proj_pieces(c):
                halves = CH_ROWS[c] // 512
                out = []
                for h in range(halves):
                    out.append(lambda h=h: p1_qk_mm(c, h))
                    out.append(lambda h=h: p1_qk_copy(c, h))
                out.append(lambda: p1_v_mm(c))
                out.append(lambda: p1_v_cp1(c))
                out.append(lambda: p1_v_tr(c))
                out.append(lambda: p1_v_cp2(c))
                return out

            # ---------------- phase-2 emitters ----------------
            o_tiles = {}

            def emit_qk(qc, k):
                st = stp.tile([P, WIN, QC], f32, tag="st", name="st")
                if heater:
                    nc.tensor.matmul(st[:, 0, 0:P], heat, heat,
                                     start=True, stop=True)
                for j in range(WIN):
                    kj = k + j
                    hp = (kj % 2) * H
                    # k rows: partitions 64:128 of qkT, 0:64 of qkTs;
                    # q rows: partitions 0:64 of qkT, 64:128 of qkTs.
                    kt = qkTs if hp == 0 else qkT
                    qt = qkT if hp == 0 else qkTs
                    nc.tensor.matmul(
                        st[:, j],
                        kt[hp:hp + H, kj * P:(kj + 1) * P],
                        qt[hp:hp + H, qc * QC:(qc + 1) * QC],
                        start=True, stop=True,
                        tile_position=(hp, 0),
                    )
                return st

            def emit_exp(st, use_dve):
                pt = ptp.tile([P, WIN, QC], bf16, tag="pt", name="pt")
                if use_dve:
                    nc.vector._custom_dve(
                        EXP2, out=pt.bitcast(u16), in0=st, in1=c3t,
                        s0=E2_C0, s1=E2_C1, imm2=E2_C2,
                    )
                else:
                    nc.scalar.activation(
                        pt, st, mybir.ActivationFunctionType.Exp,
                        bias=exp_bias, scale=float(np.log(2.0) / 128.0),
                    )
                return pt

            RESUME_K = 16          # k-chunk where partially-drained qcs resume
            part_sb = persist.tile([H + 1, 4, QC], f32)   # p1 partial PV sums
            resumed = set()

            def emit_pv_group(group):
                # j-major across the group so the v_aug stationary is shared
                for (qc, k, pt) in group:
                    if k == 0 or (qc in resumed and k == RESUME_K):
                        o_tiles[qc] = opp.tile([P, QC], f32, tag="o",
                                               name="o_ps")
                for j in range(WIN):
                    for (qc, k, pt) in group:
                        start = (k + j == 0) or (
                            qc in resumed and k + j == RESUME_K)
                        nc.tensor.matmul(
                            o_tiles[qc][0:VA], v_aug[:, k + j], pt[:, j],
                            start=start, stop=(k + j == KC - 1),
                            skip_group_check=True,
                        )

            def emit_partial_drain(qc):
                o_ps = o_tiles.pop(qc)
                nc.vector.tensor_copy(part_sb[:, qc % 4], o_ps[0:H + 1])
                resumed.add(qc)

            def emit_drain(qc):
                o_ps = o_tiles.pop(qc)
                oT = drainp.tile([H + 1, QC], f32, tag="oT", name="oT")
                if qc in resumed:
                    nc.vector.tensor_add(oT, o_ps[0:H + 1], part_sb[:, qc % 4])
                else:
                    nc.vector.tensor_copy(oT, o_ps[0:H + 1])
                t_ps = stp.tile([P, WIN, QC], f32, tag="st", name="t_ps")
                tps = t_ps.rearrange("p a b -> p (a b)")[
                    :, 0:(QC // P) * (H + 1)
                ].rearrange("p (j h) -> p j h", h=H + 1)
                if heater:
                    nc.tensor.matmul(
                        t_ps.rearrange("p a b -> p (a b)")[:, 0:P],
                        heat, heat, start=True, stop=True,
                    )
                stage = drainp.tile([P, QC // P, H], f32, tag="stage",
                                    name="stage")
                rz = drainp.tile([P, QC // P, 1], f32, tag="rz", name="rz")
                for j in range(QC // P):
                    nc.tensor.transpose(
                        tps[:, j], oT[:, j * P:(j + 1) * P],
                        ident[:H + 1, :H + 1],
                    )
                nc.vector.reciprocal(rz, tps[:, :, H:H + 1])
                for j in range(QC // P):
                    nc.vector.tensor_scalar_mul(
                        stage[:, j], tps[:, j, 0:H], rz[:, j]
                    )
                nc.sync.dma_start(
                    out_d[qc * QC:(qc + 1) * QC, :].rearrange(
                        "(j p) h -> p j h", p=P
                    ),
                    stage,
                )

            # ---------------- schedule ----------------
            # Hand-rolled era plan.  k-availability follows the p1 chunks
            # [512,512,1024,1024,1024] -> k-chunks [4,8,16,24,32].  During p1,
            # qc0..3 each accumulate k<16 into PSUM and partially drain to
            # SBUF (only 2 PSUM o-banks exist), resuming k>=16 later; this
            # doubles the window work available to hide the serialized x^T
            # DMA transposes.  Windows of a qc pair share k so their PVs can
            # be emitted j-major with a shared v_aug stationary.
            def zipk(qcs, k0, k1):
                return [(qc, k) for k in range(k0, k1, WIN) for qc in qcs]

            windows = []       # (qc, k)
            pre_actions = {}   # idx -> thunks before emit_qk
            post_actions = {}  # idx -> thunks after emit_qk

            def at_start(era_idx, thunk):
                pre_actions.setdefault(era_idx, []).append(thunk)

            def at_tail(era_start, era_end, pieces):
                n = len(pieces)
                for pi, piece in enumerate(pieces):
                    idx = max(era_start, era_end - n + pi)
                    post_actions.setdefault(idx, []).append(piece)

            # era0 (chunk0 ready): qc0 k<4
            windows += zipk([0], 0, 4)
            at_start(0, lambda: p1_dma(1))
            at_tail(0, len(windows), p1_proj_pieces(1))
            # era1 (chunk1): qc0 k4-8, qc1 k<8
            e1 = len(windows)
            windows += [(0, 4), (1, 0), (0, 6), (1, 2), (1, 4), (1, 6)]
            at_start(e1, lambda: p1_dma(2))
            at_tail(e1, len(windows), p1_proj_pieces(2))
            # era2 (chunk2): qc0,1 k8-16; partial-drain 0,1; qc2,3 k<16
            e2 = len(windows)
            windows += zipk([0, 1], 8, 16)
            at_start(e2, lambda: p1_dma(3))
            pd01 = len(windows)  # after the pv of these windows: partials
            windows += zipk([2, 3], 0, 16)
            at_start(len(windows) - 8, lambda: p1_dma(4))
            at_tail(e2, len(windows), p1_proj_pieces(3))
            # era3 (chunk3): qc0,1 resume k16-24
            e3 = len(windows)
            windows += zipk([0, 1], 16, 24)
            pd23 = e3           # qc2,3 partial-drained once era3 starts
            at_tail(e3, len(windows), p1_proj_pieces(4))
            # era4 (chunk4): qc0,1 k24-32 (finishes qc0,1)
            windows += zipk([0, 1], 24, 32)
            # post-p1: qc2,3 resume; then pairs (4,5), (6,7)
            windows += zipk([2, 3], 16, 32)
            windows += zipk([4, 5], 0, 32)
            windows += zipk([6, 7], 0, 32)
            assert len(windows) == NQC * KC // WIN

            # partial-drain after the PV of the last k<16 window of each qc
            partial_after = {pd01 - 1: [0, 1], pd23 - 1: [2, 3]}

            with nc.named_scope("p1_c0"):
                p1_dma(0)
                for piece in p1_proj_pieces(0):
                    piece()

            # ---- software pipeline over window groups ----
            n = len(windows)
            use_dve = [
                ((i + 1) * dve_num // dve_den) > (i * dve_num // dve_den)
                for i in range(n)
            ]
            # group consecutive windows sharing k (for j-major merged PV)
            groups = []
            i = 0
            while i < n:
                if (i + 1 < n and windows[i][1] == windows[i + 1][1]
                        and windows[i][0] != windows[i + 1][0]):
                    groups.append([i, i + 1])
                    i += 2
                else:
                    groups.append([i])
                    i += 1
            sts = {}
            pts = {}

            def stage_qk(i):
                qc, k = windows[i]
                with nc.named_scope(f"qk{i}_q{qc}_k{k}"):
                    for act in pre_actions.get(i, ()):
                        act()
                    sts[i] = emit_qk(qc, k)
                    for act in post_actions.get(i, ()):
                        act()

            def stage_exp(i):
                pts[i] = emit_exp(sts.pop(i), use_dve[i])

            def stage_pv_group(g):
                grp = [(windows[i][0], windows[i][1], pts.pop(i)) for i in g]
                with nc.named_scope(f"pv{g[0]}"):
                    emit_pv_group(grp)
                    for i in g:
                        qc, k = windows[i]
                        if k + WIN == KC:
                            emit_drain(qc)
                        for pqc in partial_after.get(i, ()):
                            emit_partial_drain(pqc)

            ng = len(groups)
            for gi in range(ng):
                if gi >= 1:
                    for i in groups[gi - 1]:
                        stage_exp(i)
                for i in groups[gi]:
                    stage_qk(i)
                if gi >= 2:
                    stage_pv_group(groups[gi - 2])
            with nc.named_scope("p2_tail"):
                for i in groups[ng - 1]:
                    stage_exp(i)
                stage_pv_group(groups[ng - 2])
                stage_pv_group(groups[ng - 1])

    nc.compile()
    return nc


def make_host_inputs(x):
    """fp16 cast of x, feature-chunk-major so each [S, 128] slab is contiguous
    for the xbar DMA transpose. x: [..., S, D]."""
    s, d = x.shape[-2], x.shape[-1]
    lead = x.shape[:-2]
    nf = d // P
    x16 = x.astype(np.float16).reshape(*lead, s, nf, P).swapaxes(-2, -3)
    return np.ascontiguousarray(x16)


def kernel(x, W_q, W_k, W_v):
    from concourse.bass_utils import run_bass_kernel_spmd

    x = np.ascontiguousarray(np.asarray(x, dtype=np.float32))
    W_q = np.ascontiguousarray(np.asarray(W_q, dtype=np.float32) * np.float32(QBIT))
    W_k = np.ascontiguousarray(np.asarray(W_k, dtype=np.float32))
    W_v = np.ascontiguousarray(np.asarray(W_v, dtype=np.float32))

    x16 = make_host_inputs(x)

    if "nc" not in _cached:
        _cached["nc"] = build_program()
    nc = _cached["nc"]

    in_maps = [
        {"x16": x16[c], "wq": W_q, "wk": W_k, "wv": W_v}
        for c in range(B)
    ]
    res = run_bass_kernel_spmd(nc, in_maps, core_ids=list(range(B)))
    _cached["last_res"] = res
    return np.stack([r["out"] for r in res.results], axis=0)


if __name__ == "__main__":
    rng = np.random.default_rng(0)
    x = rng.standard_normal((B, S, D), dtype=np.float32)
    Wq = rng.standard_normal((D, H), dtype=np.float32) * D ** -0.5
    Wk = rng.standard_normal((D, H), dtype=np.float32) * D ** -0.5
    Wv = rng.standard_normal((D, H), dtype=np.float32) * D ** -0.5
    out = kernel(x, Wq, Wk, Wv)
    print(out.shape, out.dtype)


# revision 14
# speedup vs baseline: 1.0707x; 1.0707x over previous
"""Trainium2 Bass kernel for nn_CausalAttention (no actual causal mask, per the
reference bug): out = softmax((x@Wq)(x@Wk)^T / 64**0.05) @ (x@Wv).

Sharding: data-parallel over batch, one batch element per NeuronCore (B=8).

Key structure (v2 — dual-engine softmax + phase overlap):
 - Scores are produced in "bf16-bit units": W_q is pre-scaled on the HOST by
   128*log2(e)/SCALE, so the QK^T PSUM value s_b satisfies
   exp(s/SCALE) = 2^(s_b/128).  The exp is then computed on TWO engines
   concurrently (windows split between them):
     * ACT (scalar) engine: exact spline exp with scale=ln2/128, bias=-25.
     * DVE (vector) engine: a CUSTOM 8-stage ALU op (EXP2_BITS_ANT) that
       computes the bf16 BIT PATTERN of 2^(s_b/128 + c) directly:
       magic-number floor-split + quadratic mantissa-hump correction,
       written as uint16 and bit-viewed as bf16 (max rel err ~0.6%, on par
       with the bf16 quantization the exact path pays anyway).
   The scalar engine alone was the phase-2 bottleneck (100% busy at 205us).
 - 2-deep software pipeline: the PE FIFO runs QK(i+2) before PV(i), so
   exp(i) and exp(i+1) run concurrently on the two engines while the PE
   streams the previous window's PV.
 - Phase 1 (x^T DMA transposes + q/k/v projections) is interleaved into the
   first q-chunk pair's windows so the serialized xbar transposes (~1.35us
   per [1024,128] fp16 slab, one safe DMA ring) hide behind attention.
   The first s-chunk is 512 rows to shorten the startup ramp.
 - q^T/k^T are produced in ONE projection pass with stationary [Wq|Wk]
   (M=128); PSUM->SBUF copies go to the SCALAR engine, and the swapped-half
   copy needed for row-paired QK^T matmuls (two K=64 matmuls in the two PE
   row halves via tile_position) runs on GPSIMD — the DVE stays free for exp.
 - v_aug is padded to M=128 ([v | ones | zeros]) so PV matmuls are
   full-array (K=128, M=128) and keep the PE HAM clock-gate at 2.4 GHz
   without heater matmuls.
 - probabilities bf16, x fp16 (host-preformatted feature-chunk-major for the
   2-byte xbar DMA transpose), all matmuls accumulate in fp32 PSUM; softmax
   denominator comes free via the ones column (sum of the same rounded
   weights -> output stays a proper weighted average).
"""

import sys

import numpy as np

for _p in ("/root/.axon_site", "/root/.axon_site/_ro/trn_rl_repo",
           "/root/.axon_site/_ro/pypackages", "/opt/trn_rl_repo"):
    if _p not in sys.path:
        sys.path.append(_p)

B, S, D, H = 8, 4096, 768, 64
P = 128
SCALE = float(H) ** 0.05
LOG2E = 1.4426950408889634
QBIT = 128.0 * LOG2E / SCALE      # host pre-scale folded into W_q
EXP_SHIFT = -25.0                 # common shift, cancels in softmax

# EXP2_BITS_ANT constants (fp32->uint16 cast rounds to nearest, HW-verified)
E2_C2 = 0.00265                                      # mantissa-hump quad coef
E2_C0 = 128.0 * (127.0 + EXP_SHIFT * LOG2E) - 64.0   # bias - 64 (floor split)
E2_C1 = 1.5 * 2.0 ** 30                              # magic (rounds to k*128)
E2_C3 = (64.0 - 4096.0 * E2_C2) / E2_C2              # alignment const / coef

_cached = {}


def _register_exp2():
    """Register the custom DVE op computing bf16 bits of 2^((x+C0)/128)+hump.

    bits = Y2 + ((F*F) + C3)*C2 ; Y2 = x + C0 ; F = Y2 - round128(Y2)
    Exactly 8 ALU stages; C3 spilled via in1 per the custom-DVE API.
    """
    from concourse import dve_ops as dvo
    from concourse.dve_spec import (
        C0, C1, C2, C3, Spec, Src0, _spill_c3_to_src1, lower,
    )
    from concourse.dve_uop import DveOpSpec

    for op in dvo.OPS:
        if op.name == "EXP2_BITS_ANT":
            return op

    Y2 = Src0 + C0
    T = Y2 + C1
    N = T - C1
    F = Y2 - N
    Q = F * F
    bits = Y2 + (Q + C3) * C2

    def ref(in0, in1, c0, c1, c2):
        y2 = in0.astype(np.float32) + np.float32(c0)
        t = (y2 + np.float32(c1)).astype(np.float32)
        n = (t - np.float32(c1)).astype(np.float32)
        f = y2 - n
        c3 = np.asarray(in1, np.float32).reshape(in0.shape[0], 1)
        return ((f * f) + c3) * np.float32(c2) + y2

    spec = Spec(body=_spill_c3_to_src1(bits), reference=ref)
    shas = {}
    for ver in ("v3", "v4"):
        uops = lower(spec, ver=ver)
        shas[ver] = DveOpSpec(
            name="EXP2_BITS_ANT", opcode=None, uops=uops, rd1_en=True
        ).sha(ver)
    op = dvo.DveOp("EXP2_BITS_ANT", spec, subdim=False, uops_sha=shas)
    dvo.OPS.append(op)
    dvo.CUSTOM_DVE_SPECS[op.name] = op.spec
    dvo._SUB_OPCODE_FOR_NAME[op.name] = dvo._CUSTOM_DVE_ROW_BASE + len(dvo.OPS) - 1
    return op


def build_program(S=S, D=D, H=H, QC=512, WIN=2, dve_num=1, dve_den=2,
                  heater=False, va=P, qk_copy_eng="scalar", swap_eng="gpsimd"):
    import concourse.mybir as mybir
    import concourse.tile as tile
    from concourse import bacc
    from concourse.masks import make_identity

    EXP2 = _register_exp2()

    NF = D // P          # feature chunks (6)
    KC = S // P          # k-chunks (32)
    NQC = S // QC        # q-chunks (8)
    VA = va              # padded v_aug width (full-array PV)
    # phase-1 s-chunks (rows); first ones smaller for a fast startup ramp
    CH_ROWS = [512, 512, 1024, 1024, 1024]
    assert sum(CH_ROWS) == S
    CH_K = [r // P for r in CH_ROWS]          # k-chunks per p1 chunk
    CH_OFF = [sum(CH_ROWS[:i]) for i in range(len(CH_ROWS))]

    f32 = mybir.dt.float32
    f16 = mybir.dt.float16
    bf16 = mybir.dt.bfloat16
    u16 = mybir.dt.uint16

    nc = bacc.Bacc("TRN2", target_bir_lowering=False)

    x_d = nc.dram_tensor("x16", [NF, P, S], f16, kind="ExternalInput")
    wq_d = nc.dram_tensor("wq", [D, H], f32, kind="ExternalInput")  # pre-scaled
    wk_d = nc.dram_tensor("wk", [D, H], f32, kind="ExternalInput")
    wv_d = nc.dram_tensor("wv", [D, H], f32, kind="ExternalInput")
    out_d = nc.dram_tensor("out", [S, H], f32, kind="ExternalOutput")

    with tile.TileContext(nc) as tc:
        with (
            tc.tile_pool(name="persist", bufs=1) as persist,
            tc.tile_pool(name="xts", bufs=2) as xts,
            tc.tile_pool(name="ptp", bufs=4) as ptp,
            tc.tile_pool(name="drainp", bufs=2) as drainp,
            tc.tile_pool(name="stp", bufs=3, space="PSUM") as stp,
            tc.tile_pool(name="op", bufs=2, space="PSUM") as opp,
        ):
            # [q (0:64) ; k (64:128)] on partitions, s on free dim
            qkT = persist.tile([P, S], f16)
            qkTs = persist.tile([P, S], f16)      # halves swapped: [k ; q]
            v_aug = persist.tile([P, KC, VA], f16)  # [kpart, chunk, v|1|0pad]
            w_stage = persist.tile([P, 3, NF, H], f32)
            wqk_sb = persist.tile([P, NF, P], f16)  # [ Wq | Wk ] per chunk
            wv_sb = persist.tile([P, NF, H], f16)
            ident = persist.tile([P, P], f32)
            ident16 = persist.tile([P, P], f16)
            exp_bias = persist.tile([P, 1], f32)
            c3t = persist.tile([P, 1], f32)
            heat = persist.tile([P, P], f16)

            make_identity(nc, ident)
            make_identity(nc, ident16)
            if VA > H + 1:
                nc.vector.memset(v_aug[:, :, H:VA], 0.0)
            nc.vector.memset(v_aug[:, :, H:H + 1], 1.0)
            nc.vector.memset(exp_bias, EXP_SHIFT)
            nc.vector.memset(c3t, E2_C3)
            if heater:
                nc.vector.memset(heat, 0.001)
            for i, w_d in enumerate((wq_d, wk_d, wv_d)):
                nc.sync.dma_start(
                    w_stage[:, i], w_d[:].rearrange("(g p) h -> p g h", p=P)
                )
            nc.vector.tensor_copy(wqk_sb[:, :, 0:H], w_stage[:, 0])
            nc.vector.tensor_copy(wqk_sb[:, :, H:P], w_stage[:, 1])
            nc.vector.tensor_copy(wv_sb[:], w_stage[:, 2])

            qk_copy = nc.scalar.copy if qk_copy_eng == "scalar" else \
                (lambda o, i_: nc.vector.tensor_copy(o, i_))
            swap_copy = nc.gpsimd.tensor_copy if swap_eng == "gpsimd" else \
                nc.vector.tensor_copy

            # ---------------- phase-1 pieces ----------------
            xf_tiles = {}

            def p1_dma(c):
                rows = CH_ROWS[c]
                sl = slice(CH_OFF[c], CH_OFF[c] + rows)
                xf = xts.tile([P, NF, 1024], f16, tag="xf", name="xf")
                for g in range(NF):
                    nc.sync.dma_start(xf[:, g, 0:rows], x_d[g, :, sl])
                xf_tiles[c] = xf

            qk_ps = {}

            def p1_qk_mm(c, half):
                xf = xf_tiles[c]
                hs = slice(half * 512, (half + 1) * 512)
                ps = stp.tile([P, WIN, QC], f32, tag="st", name="ps")
                psf = ps.rearrange("p a b -> p (a b)")
                for g in range(NF):
                    nc.tensor.matmul(
                        psf[:, 0:512], wqk_sb[:, g], xf[:, g, hs],
                        start=(g == 0), stop=(g == NF - 1),
                    )
                qk_ps[(c, half)] = psf

            def p1_qk_copy(c, half):
                psf = qk_ps.pop((c, half))
                col0 = CH_OFF[c] + half * 512
                cols = slice(col0, col0 + 512)
                qk_copy(qkT[:, cols], psf[:, 0:512])
                # swapped halves [k ; q] for the paired QK^T matmuls
                swap_copy(qkTs[0:H, cols], qkT[H:P, cols])
                swap_copy(qkTs[H:P, cols], qkT[0:H, cols])

            v_ps = {}

            def p1_v_mm(c):
                # v^T = Wv^T x^T with the CONSTANT wv stationary: 6 stream-
                # bound matmuls instead of 48 LDW-bound ones per chunk.
                xf = xf_tiles[c]
                rows = CH_ROWS[c]
                ps = stp.tile([P, WIN, QC], f32, tag="st", name="psvt")
                psf = ps.rearrange("p a b -> p (a b)")
                for half in range(rows // 512):
                    hs = slice(half * 512, (half + 1) * 512)
                    for g in range(NF):
                        nc.tensor.matmul(
                            psf[0:H, hs], wv_sb[:, g], xf[:, g, hs],
                            start=(g == 0), stop=(g == NF - 1),
                        )
                v_ps[c] = psf
                xf_tiles.pop(c)

            def p1_v_cp1(c):
                psf = v_ps.pop(c)
                rows = CH_ROWS[c]
                vt = drainp.tile([H, 1024], f16, tag="vts", name="vt")
                nc.scalar.copy(vt[:, 0:rows], psf[0:H, 0:rows])
                v_ps[c] = vt

            def p1_v_tr(c):
                vt = v_ps.pop(c)
                kpc = CH_K[c]
                ps2 = stp.tile([P, WIN * QC], f16, tag="st", name="psv2")
                for t in range(kpc):
                    nc.tensor.transpose(
                        ps2[:, t * H:(t + 1) * H],
                        vt[:, t * P:(t + 1) * P],
                        ident16[0:H, 0:H],
                    )
                v_ps[c] = ps2

            def p1_v_cp2(c):
                ps2 = v_ps.pop(c)
                kpc = CH_K[c]
                k0 = CH_OFF[c] // P
                src_v = ps2[:, 0:kpc * H].rearrange("p (t h) -> p t h", h=H)
                nc.vector.tensor_copy(v_aug[:, k0:k0 + kpc, 0:H], src_v)

            def p1_proj_pieces(c):
                halves = CH_ROWS[c] // 512
                out = []
                for h in range(halves):
                    out.append(lambda h=h: p1_qk_mm(c, h))
                    out.append(lambda h=h: p1_qk_copy(c, h))
                out.append(lambda: p1_v_mm(c))
                out.append(lambda: p1_v_cp1(c))
                out.append(lambda: p1_v_tr(c))
                out.append(lambda: p1_v_cp2(c))
                return out

            # ---------------- phase-2 emitters ----------------
            o_tiles = {}

            def emit_qk(qc, k):
                st = stp.tile([P, WIN, QC], f32, tag="st", name="st")
                if heater:
                    nc.tensor.matmul(st[:, 0, 0:P], heat, heat,
                                     start=True, stop=True)
                for j in range(WIN):
                    kj = k + j
                    hp = (kj % 2) * H
                    # k rows: partitions 64:128 of qkT, 0:64 of qkTs;
                    # q rows: partitions 0:64 of qkT, 64:128 of qkTs.
                    kt = qkTs if hp == 0 else qkT
                    qt = qkT if hp == 0 else qkTs
                    nc.tensor.matmul(
                        st[:, j],
                        kt[hp:hp + H, kj * P:(kj + 1) * P],
                        qt[hp:hp + H, qc * QC:(qc + 1) * QC],
                        start=True, stop=True,
                        tile_position=(hp, 0),
                    )
                return st

            def emit_exp(st, use_dve):
                pt = ptp.tile([P, WIN, QC], bf16, tag="pt", name="pt")
                if use_dve:
                    nc.vector._custom_dve(
                        EXP2, out=pt.bitcast(u16), in0=st, in1=c3t,
                        s0=E2_C0, s1=E2_C1, imm2=E2_C2,
                    )
                else:
                    nc.scalar.activation(
                        pt, st, mybir.ActivationFunctionType.Exp,
                        bias=exp_bias, scale=float(np.log(2.0) / 128.0),
                    )
                return pt

            RESUME_K = 16          # k-chunk where partially-drained qcs resume
            part_sb = persist.tile([H + 1, 4, QC], f32)   # p1 partial PV sums
            resumed = set()

            def emit_pv_group(group):
                # j-major across the group so the v_aug stationary is shared
                for (qc, k, pt) in group:
                    if k == 0 or (qc in resumed and k == RESUME_K):
                        o_tiles[qc] = opp.tile([P, QC], f32, tag="o",
                                               name="o_ps")
                for j in range(WIN):
                    for (qc, k, pt) in group:
                        start = (k + j == 0) or (
                            qc in resumed and k + j == RESUME_K)
                        nc.tensor.matmul(
                            o_tiles[qc][0:VA], v_aug[:, k + j], pt[:, j],
                            start=start, stop=(k + j == KC - 1),
                            skip_group_check=True,
                        )

            def emit_partial_drain(qc):
                o_ps = o_tiles.pop(qc)
                nc.vector.tensor_copy(part_sb[:, qc % 4], o_ps[0:H + 1])
                resumed.add(qc)

            def emit_drain(qc):
                o_ps = o_tiles.pop(qc)
                oT = drainp.tile([H + 1, QC], f32, tag="oT", name="oT")
                if qc in resumed:
                    nc.vector.tensor_add(oT, o_ps[0:H + 1], part_sb[:, qc % 4])
                else:
                    nc.vector.tensor_copy(oT, o_ps[0:H + 1])
                t_ps = stp.tile([P, WIN, QC], f32, tag="st", name="t_ps")
                tps = t_ps.rearrange("p a b -> p (a b)")[
                    :, 0:(QC // P) * (H + 1)
                ].rearrange("p (j h) -> p j h", h=H + 1)
                if heater:
                    nc.tensor.matmul(
                        t_ps.rearrange("p a b -> p (a b)")[:, 0:P],
                        heat, heat, start=True, stop=True,
                    )
                stage = drainp.tile([P, QC // P, H], f32, tag="stage",
                                    name="stage")
                rz = drainp.tile([P, QC // P, 1], f32, tag="rz", name="rz")
                for j in range(QC // P):
                    nc.tensor.transpose(
                        tps[:, j], oT[:, j * P:(j + 1) * P],
                        ident[:H + 1, :H + 1],
                    )
                nc.vector.reciprocal(rz, tps[:, :, H:H + 1])
                for j in range(QC // P):
                    nc.vector.tensor_scalar_mul(
                        stage[:, j], tps[:, j, 0:H], rz[:, j]
                    )
                nc.sync.dma_start(
                    out_d[qc * QC:(qc + 1) * QC, :].rearrange(
                        "(j p) h -> p j h", p=P
                    ),
                    stage,
                )

            # ---------------- schedule ----------------
            # Hand-rolled era plan.  k-availability follows the p1 chunks
            # [512,512,1024,1024,1024] -> k-chunks [4,8,16,24,32].  During p1,
            # qc0..3 each accumulate k<16 into PSUM and partially drain to
            # SBUF (only 2 PSUM o-banks exist), resuming k>=16 later; this
            # doubles the window work available to hide the serialized x^T
            # DMA transposes.  Windows of a qc pair share k so their PVs can
            # be emitted j-major with a shared v_aug stationary.
            def zipk(qcs, k0, k1):
                return [(qc, k) for k in range(k0, k1, WIN) for qc in qcs]

            windows = []       # (qc, k)
            pre_actions = {}   # idx -> thunks before emit_qk
            post_actions = {}  # idx -> thunks after emit_qk

            def at_start(era_idx, thunk):
                pre_actions.setdefault(era_idx, []).append(thunk)

            def at_tail(era_start, era_end, pieces):
                n = len(pieces)
                for pi, piece in enumerate(pieces):
                    idx = max(era_start, era_end - n + pi)
                    post_actions.setdefault(idx, []).append(piece)

            # era0 (chunk0 ready): qc0 k<4
            windows += zipk([0], 0, 4)
            at_start(0, lambda: p1_dma(1))
            at_tail(0, len(windows), p1_proj_pieces(1))
            # era1 (chunk1): qc0 k4-8, qc1 k<8
            e1 = len(windows)
            windows += [(0, 4), (1, 0), (0, 6), (1, 2), (1, 4), (1, 6)]
            at_start(e1, lambda: p1_dma(2))
            at_tail(e1, len(windows), p1_proj_pieces(2))
            # era2 (chunk2): qc0,1 k8-16; partial-drain 0,1; qc2,3 k<16
            e2 = len(windows)
            windows += zipk([0, 1], 8, 16)
            at_start(e2, lambda: p1_dma(3))
            pd01 = len(windows)  # after the pv of these windows: partials
            windows += zipk([2, 3], 0, 16)
            at_start(len(windows) - 8, lambda: p1_dma(4))
            at_tail(e2, len(windows), p1_proj_pieces(3))
            # era3 (chunk3): qc0,1 resume k16-24
            e3 = len(windows)
            windows += zipk([0, 1], 16, 24)
            pd23 = e3           # qc2,3 partial-drained once era3 starts
            at_tail(e3, len(windows), p1_proj_pieces(4))
            # era4 (chunk4): qc0,1 k24-32 (finishes qc0,1)
            windows += zipk([0, 1], 24, 32)
            # post-p1: qc2,3 resume; then pairs (4,5), (6,7)
            windows += zipk([2, 3], 16, 32)
            windows += zipk([4, 5], 0, 32)
            windows += zipk([6, 7], 0, 32)
            assert len(windows) == NQC * KC // WIN

            # partial-drain after the PV of the last k<16 window of each qc
            partial_after = {pd01 - 1: [0, 1], pd23 - 1: [2, 3]}

            with nc.named_scope("p1_c0"):
                p1_dma(0)
                for piece in p1_proj_pieces(0):
                    piece()

            # ---- software pipeline over window groups ----
            n = len(windows)
            use_dve = [
                ((i + 1) * dve_num // dve_den) > (i * dve_num // dve_den)
                for i in range(n)
            ]
            # group consecutive windows sharing k (for j-major merged PV)
            groups = []
            i = 0
            while i < n:
                if (i + 1 < n and windows[i][1] == windows[i + 1][1]
                        and windows[i][0] != windows[i + 1][0]):
                    groups.append([i, i + 1])
                    i += 2
                else:
                    groups.append([i])
                    i += 1
            sts = {}
            pts = {}

            def stage_qk(i):
                qc, k = windows[i]
                with nc.named_scope(f"qk{i}_q{qc}_k{k}"):
                    for act in pre_actions.get(i, ()):
                        act()
                    sts[i] = emit_qk(qc, k)
                    for act in post_actions.get(i, ()):
                        act()

            def stage_exp(i):
                pts[i] = emit_exp(sts.pop(i), use_dve[i])

            def stage_pv_group(g):
                grp = [(windows[i][0], windows[i][1], pts.pop(i)) for i in g]
                with nc.named_scope(f"pv{g[0]}"):
                    emit_pv_group(grp)
                    for i in g:
                        qc, k = windows[i]
                        if k + WIN == KC:
                            emit_drain(qc)
                        for pqc in partial_after.get(i, ()):
                            emit_partial_drain(pqc)

            ng = len(groups)
            for gi in range(ng):
                if gi >= 1:
                    for i in groups[gi - 1]:
                        stage_exp(i)
                for i in groups[gi]:
                    stage_qk(i)
                if gi >= 2:
                    stage_pv_group(groups[gi - 2])
            with nc.named_scope("p2_tail"):
                for i in groups[ng - 1]:
                    stage_exp(i)
                stage_pv_group(groups[ng - 2])
                stage_pv_group(groups[ng - 1])

    nc.compile()
    return nc


def make_host_inputs(x):
    """fp16 cast of x, pre-transposed on the host to [..., NF, 128, S] so the
    device needs only plain linear DMAs (no xbar transposes). x: [..., S, D]."""
    s, d = x.shape[-2], x.shape[-1]
    lead = x.shape[:-2]
    nf = d // P
    x16 = x.astype(np.float16).reshape(*lead, s, nf, P).swapaxes(-2, -3)
    x16 = np.swapaxes(x16, -1, -2)   # [..., nf, P, S]
    return np.ascontiguousarray(x16)


def kernel(x, W_q, W_k, W_v):
    from concourse.bass_utils import run_bass_kernel_spmd

    x = np.ascontiguousarray(np.asarray(x, dtype=np.float32))
    W_q = np.ascontiguousarray(np.asarray(W_q, dtype=np.float32) * np.float32(QBIT))
    W_k = np.ascontiguousarray(np.asarray(W_k, dtype=np.float32))
    W_v = np.ascontiguousarray(np.asarray(W_v, dtype=np.float32))

    x16 = make_host_inputs(x)

    if "nc" not in _cached:
        _cached["nc"] = build_program()
    nc = _cached["nc"]

    in_maps = [
        {"x16": x16[c], "wq": W_q, "wk": W_k, "wv": W_v}
        for c in range(B)
    ]
    res = run_bass_kernel_spmd(nc, in_maps, core_ids=list(range(B)))
    _cached["last_res"] = res
    return np.stack([r["out"] for r in res.results], axis=0)


if __name__ == "__main__":
    rng = np.random.default_rng(0)
    x = rng.standard_normal((B, S, D), dtype=np.float32)
    Wq = rng.standard_normal((D, H), dtype=np.float32) * D ** -0.5
    Wk = rng.standard_normal((D, H), dtype=np.float32) * D ** -0.5
    Wv = rng.standard_normal((D, H), dtype=np.float32) * D ** -0.5
    out = kernel(x, Wq, Wk, Wv)
    print(out.shape, out.dtype)
